# revision 10
# baseline (speedup 1.0000x reference)
"""Trainium2 Bass kernel for the ActorCritic LSTM scan.

Problem: B=128, T=2048, HIDDEN=256, IN_DIM=82 LSTM (torch.nn.LSTMCell
semantics, gate order i,f,g,o) with the input projection hoisted out of
the recurrence.

Strategy (8 NeuronCores, data-parallel over batch, 16 rows per core):
  - Gate-on-partition layout: the per-step gate matmul produces
    gates.T [1024, 16] as 8 partition-tiles of [128, 16] living in the
    free dim of one PSUM chunk; batch lives on the free dim so the
    ScalarE/VectorE per-step ops have tiny free-dim sizes.
  - The input projection xp = W_ih @ feats.T (+ biases) is computed in
    bulk per 8-step chunk directly into PSUM via one-hot matmuls; the
    recurrent matmuls then accumulate on top (start=False), so no
    separate "add xp" instruction is needed.
  - One-hot features are built on-chip: a tiny [5,82] matmul broadcasts
    (x, a, r, d, 1) rows across 82 partitions, then a per-partition
    is_equal against an iota column produces the one-hots exactly.
  - W_hh stationary tiles are [128, 32] column-group tiles issued with
    tile_position=(0, 32g) so 4 LDWEIGHTS/matmul streams run
    concurrently in the 4 column groups of the PE array.
"""

import os
import numpy as np

os.environ.setdefault("NEURON_COMPILE_CACHE_URL", "/tmp/neuron_cache")

NUM_STATES = 64
NUM_ACTIONS = 16
HIDDEN = 256
IN_DIM = NUM_STATES + NUM_ACTIONS + 2  # 82
B, T_FULL = 128, 2048
NCORES = 8
BL = B // NCORES  # 16 batch rows per core
G4 = 4 * HIDDEN  # 1024 gate rows
MT = G4 // 128  # 8 m-tiles
KT = HIDDEN // 128  # 2 k-tiles
KF = 83  # feature contraction dim (82 feats + bias/ones row)

# torch gate order i,f,g,o -> our m-tile order i,f,o,g
_PERM = np.concatenate(
    [np.arange(0, 512), np.arange(768, 1024), np.arange(512, 768)]
)


def _build(T, C=8, debug=False, coltile=True):
    """Build the Bass/Tile program for a T-step scan, C steps per chunk."""
    import concourse.bass as bass
    import concourse.tile as tile
    from concourse import bacc, mybir

    assert T % C == 0
    NCH = T // C
    NC16 = C * BL  # free width of one chunk (t,b)

    nc = bacc.Bacc("TRN2", target_bir_lowering=False, debug=debug)
    f32 = mybir.dt.float32

    inp4 = nc.declare_dram_parameter("inp4", [NCH, 5, NC16], f32, isOutput=False)
    h0c0 = nc.declare_dram_parameter("h0c0", [128, 4, BL], f32, isOutput=False)
    wh = nc.declare_dram_parameter("wh", [HIDDEN, G4], f32, isOutput=False)
    wx = nc.declare_dram_parameter("wx", [KF, G4], f32, isOutput=False)
    em = nc.declare_dram_parameter("em", [5, 80], f32, isOutput=False)
    iota = nc.declare_dram_parameter("iota", [80, 1], f32, isOutput=False)
    feat = nc.declare_dram_parameter(
        "feat", [NCH, 128, C, 2, BL], f32, isOutput=True
    )
    fh = nc.declare_dram_parameter("fh", [128, 4, BL], f32, isOutput=True)

    sig_f = mybir.ActivationFunctionType.Sigmoid
    tanh_f = mybir.ActivationFunctionType.Tanh

    with tile.TileContext(nc) as tc:
        with (
            tc.tile_pool(name="state", bufs=1) as state,
            tc.tile_pool(name="stage", bufs=3) as stage,
            tc.tile_pool(name="sv", bufs=2) as sv,
            tc.tile_pool(name="xp", bufs=2, space="PSUM") as xpp,
            tc.tile_pool(name="bc", bufs=2, space="PSUM") as bcp,
        ):
            # --- constants / persistent state ---
            wh_sb = state.tile([128, KT, G4], f32, tag="wh_sb")
            nc.sync.dma_start(
                out=wh_sb, in_=wh.rearrange("(k p) j -> p k j", p=128)
            )
            wx_sb = state.tile([KF, G4], f32, tag="wx_sb")
            nc.sync.dma_start(out=wx_sb, in_=wx[:, :])
            em_sb = state.tile([5, 80], f32, tag="em_sb")
            nc.sync.dma_start(out=em_sb, in_=em[:, :])
            iota_sb = state.tile([80, 1], f32, tag="iota_sb")
            nc.sync.dma_start(out=iota_sb, in_=iota[:, :])

            hc0 = state.tile([128, 4, BL], f32, tag="hc0")
            nc.sync.dma_start(out=hc0, in_=h0c0[:, :, :])
            c_t = state.tile([128, 2, BL], f32, tag="c_t")
            nc.vector.tensor_copy(c_t, hc0[:, 2:4, :])

            # h ring buffers (ping-pong per chunk)
            rings = [
                state.tile([128, C, 2, BL], f32, name=f"ring{i}", tag=f"ring{i}")
                for i in range(2)
            ]
            # one-hot feature staging (ping-pong per chunk)
            feats = [
                state.tile([KF, NC16], f32, name=f"feats{i}", tag=f"feats{i}")
                for i in range(2)
            ]

            def prep_chunk(c):
                """Stage inputs + build one-hot feats + xp matmuls for chunk c."""
                ft = feats[c % 2]
                inp_sb = stage.tile([5, NC16], f32, tag="inp_sb")
                nc.sync.dma_start(out=inp_sb, in_=inp4[c, :, :])
                # r, d, ones rows go straight into feats rows 80..82 via DMA
                nc.sync.dma_start(out=ft[80:83, :], in_=inp4[c, 2:5, :])
                bc = bcp.tile([80, NC16], f32, tag="bc")
                nc.tensor.matmul(bc, em_sb, inp_sb, start=True, stop=True)
                nc.vector.tensor_scalar(
                    out=ft[0:80, :],
                    in0=bc[0:80, :],
                    scalar1=iota_sb[0:80, :],
                    scalar2=None,
                    op0=mybir.AluOpType.is_equal,
                )
                xp = xpp.tile([128, MT, C, BL], f32, tag="xp")
                # start=True clears has_written for the whole PSUM bank, so
                # only the first m-tile touching each bank may set it.
                m_per_bank = 512 // (C * BL)
                for m in range(MT):
                    nc.tensor.matmul(
                        xp[:, m],
                        wx_sb[:, 128 * m : 128 * (m + 1)],
                        ft,
                        start=(m % m_per_bank == 0),
                        stop=False,
                        skip_group_check=True,
                    )
                return xp

            xp_cur = prep_chunk(0)

            for c in range(NCH):
                ring = rings[c % 2]
                xp = xp_cur
                for tl in range(C):
                    t = c * C + tl
                    # h(t-1) source
                    if t == 0:
                        h_prev = hc0[:, 0:2, :]
                    elif tl == 0:
                        h_prev = rings[(c - 1) % 2][:, C - 1]
                    else:
                        h_prev = ring[:, tl - 1]

                    # recurrent matmuls accumulate onto xp in PSUM
                    for m in range(MT):
                        if coltile:
                            for g in range(4):
                                for k in range(KT):
                                    nc.tensor.matmul(
                                        xp[32 * g : 32 * (g + 1), m, tl],
                                        wh_sb[
                                            :, k, 128 * m + 32 * g : 128 * m + 32 * (g + 1)
                                        ],
                                        h_prev[:, k, :],
                                        start=False,
                                        stop=(k == KT - 1),
                                        skip_group_check=True,
                                        tile_position=(0, 32 * g),
                                    )
                        else:
                            for k in range(KT):
                                nc.tensor.matmul(
                                    xp[:, m, tl],
                                    wh_sb[:, k, 128 * m : 128 * (m + 1)],
                                    h_prev[:, k, :],
                                    start=False,
                                    stop=(k == KT - 1),
                                    skip_group_check=True,
                                )

                    # nonlinearities + state update
                    sg = sv.tile([128, 3, 2, BL], f32, tag="sg")
                    nc.scalar.activation(sg, xp[:, 0:6, tl], sig_f)
                    tg = sv.tile([128, 2, BL], f32, tag="tg")
                    nc.scalar.activation(tg, xp[:, 6:8, tl], tanh_f)
                    fc = sv.tile([128, 2, BL], f32, tag="fc")
                    nc.vector.tensor_mul(fc, sg[:, 1], c_t)
                    ig = sv.tile([128, 2, BL], f32, tag="ig")
                    nc.vector.tensor_mul(ig, sg[:, 0], tg)
                    nc.vector.tensor_add(c_t, fc, ig)
                    tc_ = sv.tile([128, 2, BL], f32, tag="tc_")
                    nc.scalar.activation(tc_, c_t, tanh_f)
                    nc.vector.tensor_mul(ring[:, tl], sg[:, 2], tc_)

                # prefetch next chunk's xp while this chunk's tail still runs
                if c + 1 < NCH:
                    xp_cur = prep_chunk(c + 1)
                nc.sync.dma_start(out=feat[c], in_=ring)

            # final hidden: pack [h_T ; c_T]
            fh_sb = state.tile([128, 4, BL], f32, tag="fh_sb")
            nc.vector.tensor_copy(fh_sb[:, 0:2, :], rings[(NCH - 1) % 2][:, C - 1])
            nc.vector.tensor_copy(fh_sb[:, 2:4, :], c_t)
            nc.sync.dma_start(out=fh[:, :, :], in_=fh_sb)

    nc.compile()
    return nc


_NC_CACHE = {}


def _get_nc(T, C=8, debug=False, coltile=True):
    key = (T, C, debug, coltile)
    if key not in _NC_CACHE:
        _NC_CACHE[key] = _build(T, C, debug=debug, coltile=coltile)
    return _NC_CACHE[key]


def _pack_inputs(x, hidden, prev_action, prev_reward, prev_done, W_ih, W_hh,
                 b_ih, b_hh, T, C):
    """Host-side packing into the kernel's layouts. Returns per-core in_maps."""
    x = np.asarray(x, dtype=np.float32)
    a = np.asarray(prev_action, dtype=np.float32)
    r = np.asarray(prev_reward, dtype=np.float32)
    d = np.asarray(prev_done, dtype=np.float32)
    hidden = np.asarray(hidden, dtype=np.float32)
    W_ih = np.asarray(W_ih, dtype=np.float32)
    W_hh = np.asarray(W_hh, dtype=np.float32)
    bias = np.asarray(b_ih, dtype=np.float32) + np.asarray(b_hh, dtype=np.float32)

    NCH = T // C
    wh_l = np.ascontiguousarray(W_hh[_PERM, :].T)  # [256, 1024]
    wx_l = np.zeros((KF, G4), dtype=np.float32)
    wx_l[:IN_DIM, :] = W_ih[_PERM, :].T
    wx_l[IN_DIM, :] = bias[_PERM]

    em = np.zeros((5, 80), dtype=np.float32)
    em[0, :NUM_STATES] = 1.0
    em[1, NUM_STATES:NUM_STATES + NUM_ACTIONS] = 1.0

    iota = np.zeros((80, 1), dtype=np.float32)
    iota[:NUM_STATES, 0] = np.arange(NUM_STATES)
    iota[NUM_STATES:80, 0] = np.arange(NUM_ACTIONS)

    in_maps = []
    for i in range(NCORES):
        bs = slice(BL * i, BL * (i + 1))
        # inp4[c, row, tl*BL + b]; rows: x, a, r, d, ones  (values at t=c*C+tl)
        inp4 = np.empty((NCH, 5, C * BL), dtype=np.float32)
        for row, src in enumerate((x, a, r, d)):
            # src[bs] is [BL, T] -> [NCH, C, BL] with (t-major, b-minor)
            v = src[bs, :T].T.reshape(NCH, C, BL)
            inp4[:, row, :] = v.reshape(NCH, C * BL)
        inp4[:, 4, :] = 1.0

        # h0c0[p, j, b]: j=0,1 -> h halves; j=2,3 -> c halves
        hc = hidden[bs]  # [BL, 512]
        h0c0 = np.ascontiguousarray(
            hc.reshape(BL, 4, 128).transpose(2, 1, 0)
        )  # [128, 4, BL]

        in_maps.append({
            "inp4": inp4, "h0c0": h0c0, "wh": wh_l, "wx": wx_l,
            "em": em, "iota": iota,
        })
    return in_maps


def _unpack_outputs(results, T, C, x_dtype):
    NCH = T // C
    features = np.empty((B, T, HIDDEN), dtype=np.float32)
    final_hidden = np.empty((B, 2 * HIDDEN), dtype=np.float32)
    for i, res in enumerate(results):
        bs = slice(BL * i, BL * (i + 1))
        f = res["feat"]  # [NCH, 128, C, 2, BL]
        # features[b, t, 128*mc+p] = f[c, p, tl, mc, b]
        features[bs] = f.transpose(4, 0, 2, 3, 1).reshape(BL, T, HIDDEN)
        fhc = res["fh"]  # [128, 4, BL] -> [b, 4, 128] -> [b, 512]
        final_hidden[bs] = fhc.transpose(2, 1, 0).reshape(BL, 2 * HIDDEN)
    return features, final_hidden


def kernel(x, hidden, prev_action, prev_reward, prev_done, W_ih, W_hh, b_ih,
           b_hh, _T=None, _C=8, _trace=False):
    from concourse.bass_utils import run_bass_kernel_spmd

    T = _T or T_FULL
    nc = _get_nc(T, _C)
    in_maps = _pack_inputs(x, hidden, prev_action, prev_reward, prev_done,
                           W_ih, W_hh, b_ih, b_hh, T, _C)
    out = run_bass_kernel_spmd(
        nc, in_maps, core_ids=list(range(NCORES)), trace=_trace
    )
    features, final_hidden = _unpack_outputs(out.results, T, _C, None)
    if _trace:
        return (features, final_hidden), out
    return features, final_hidden


# revision 13
# speedup vs baseline: 1.4311x; 1.4311x over previous
"""Trainium2 Bass kernel for the ActorCritic LSTM scan.

Problem: B=128, T=2048, HIDDEN=256, IN_DIM=82 LSTM (torch.nn.LSTMCell
semantics, gate order i,f,g,o) with the input projection hoisted out of
the recurrence.

Strategy (8 NeuronCores, data-parallel over batch, 16 rows per core):
  - Gate-on-partition layout: the per-step gate matmul produces
    gates.T [1024, 16] as 8 partition-tiles of [128, 16] living in the
    free dim of one PSUM chunk; batch lives on the free dim so the
    ScalarE/VectorE per-step ops have tiny free-dim sizes.
  - bf16 matmul datapath (W_hh, W_ih, one-hot feats, h): fp32 matmuls
    stream at 1/4 rate on the PE. PSUM accumulation and the c/state
    arithmetic stay fp32.
  - Full [128,128] stationary tiles (16 matmuls/step) — the PE is bound
    by per-instruction dispatch + LDWEIGHTS, so fewer, bigger stationary
    tiles win over 32-column tile_position packing.
  - All nonlinearities via ONE sigmoid table: tanh(x) = 2*sigmoid(2x)-1,
    with the 2x folded into the g-gate rows of the host-prepacked
    weights. 2 ScalarE instructions per step instead of 3.
  - The input projection xp = W_ih @ feats.T (+ biases) is computed in
    bulk per 8-step chunk directly into PSUM via one-hot matmuls; the
    recurrent matmuls then accumulate on top (start=False).
  - One-hot features built on-chip: a [5,80] matmul broadcasts (x, a)
    rows across 80 partitions, then a per-partition is_equal against an
    iota column; r/d/ones rows are DMA'd straight into the feats tile.
"""

import os
import numpy as np
import ml_dtypes

os.environ.setdefault("NEURON_COMPILE_CACHE_URL", "/tmp/neuron_cache")

BF16 = ml_dtypes.bfloat16

NUM_STATES = 64
NUM_ACTIONS = 16
HIDDEN = 256
IN_DIM = NUM_STATES + NUM_ACTIONS + 2  # 82
B, T_FULL = 128, 2048
NCORES = 8
BL = B // NCORES  # 16 batch rows per core
G4 = 4 * HIDDEN  # 1024 gate rows
MT = G4 // 128  # 8 m-tiles
KT = HIDDEN // 128  # 2 k-tiles
KF = 83  # feature contraction dim (82 feats + bias/ones row)

# torch gate order i,f,g,o -> our m-tile order i,f,o,g
_PERM = np.concatenate(
    [np.arange(0, 512), np.arange(768, 1024), np.arange(512, 768)]
)


def _build(T, C=8, debug=False, repeat=1):
    """Build the Bass/Tile program for a T-step scan, C steps per chunk.

    repeat>1 re-runs the whole scan that many times back-to-back inside
    one NEFF (identical results each pass) — used only for timing.
    """
    import concourse.bass as bass
    import concourse.tile as tile
    from concourse import bacc, mybir

    assert T % C == 0
    NCH = T // C
    NC16 = C * BL  # free width of one chunk (t,b)

    nc = bacc.Bacc("TRN2", target_bir_lowering=False, debug=debug)
    f32 = mybir.dt.float32
    bf16 = mybir.dt.bfloat16

    inp4 = nc.declare_dram_parameter("inp4", [NCH, 5, NC16], f32, isOutput=False)
    inprd = nc.declare_dram_parameter("inprd", [NCH, 3, NC16], bf16, isOutput=False)
    h0 = nc.declare_dram_parameter("h0", [128, 2, BL], bf16, isOutput=False)
    c0 = nc.declare_dram_parameter("c0", [128, 2, BL], f32, isOutput=False)
    wh = nc.declare_dram_parameter("wh", [HIDDEN, G4], bf16, isOutput=False)
    wx = nc.declare_dram_parameter("wx", [KF, G4], bf16, isOutput=False)
    em = nc.declare_dram_parameter("em", [5, 80], f32, isOutput=False)
    iota = nc.declare_dram_parameter("iota", [80, 1], f32, isOutput=False)
    feat = nc.declare_dram_parameter(
        "feat", [NCH, 128, C, 2, BL], bf16, isOutput=True
    )
    fhh = nc.declare_dram_parameter("fhh", [128, 2, BL], bf16, isOutput=True)
    fhc = nc.declare_dram_parameter("fhc", [128, 2, BL], f32, isOutput=True)

    sig_f = mybir.ActivationFunctionType.Sigmoid
    mult = mybir.AluOpType.mult
    sub = mybir.AluOpType.subtract

    with tile.TileContext(nc) as tc:
        with (
            tc.tile_pool(name="state", bufs=1) as state,
            tc.tile_pool(name="stage", bufs=3) as stage,
            tc.tile_pool(name="sv", bufs=2) as sv,
            tc.tile_pool(name="xp", bufs=2, space="PSUM") as xpp,
            tc.tile_pool(name="bc", bufs=2, space="PSUM") as bcp,
        ):
            # --- constants ---
            wh_sb = state.tile([128, KT, G4], bf16, tag="wh_sb")
            nc.sync.dma_start(
                out=wh_sb, in_=wh.rearrange("(k p) j -> p k j", p=128)
            )
            wx_sb = state.tile([KF, G4], bf16, tag="wx_sb")
            nc.sync.dma_start(out=wx_sb, in_=wx[:, :])
            em_sb = state.tile([5, 80], f32, tag="em_sb")
            nc.sync.dma_start(out=em_sb, in_=em[:, :])
            iota_sb = state.tile([80, 1], f32, tag="iota_sb")
            nc.sync.dma_start(out=iota_sb, in_=iota[:, :])

            h0_sb = state.tile([128, 2, BL], bf16, tag="h0_sb")
            nc.sync.dma_start(out=h0_sb, in_=h0[:, :, :])

            c_t = state.tile([128, 2, BL], f32, tag="c_t")
            rings = [
                state.tile([128, C, 2, BL], bf16, name=f"ring{i}", tag=f"ring{i}")
                for i in range(2)
            ]
            feats = [
                state.tile([KF, NC16], bf16, name=f"feats{i}", tag=f"feats{i}")
                for i in range(2)
            ]

            def prep_chunk(c):
                """Stage inputs + build one-hot feats + xp matmuls for chunk c."""
                ft = feats[c % 2]
                inp_sb = stage.tile([5, NC16], f32, tag="inp_sb")
                nc.sync.dma_start(out=inp_sb, in_=inp4[c, :, :])
                nc.sync.dma_start(out=ft[80:83, :], in_=inprd[c, :, :])
                bc = bcp.tile([80, NC16], f32, tag="bc")
                nc.tensor.matmul(bc, em_sb, inp_sb, start=True, stop=True)
                nc.vector.tensor_scalar(
                    out=ft[0:80, :],
                    in0=bc[0:80, :],
                    scalar1=iota_sb[0:80, :],
                    scalar2=None,
                    op0=mybir.AluOpType.is_equal,
                )
                xp = xpp.tile([128, MT, C, BL], f32, tag="xp")
                # start=True clears has_written for the whole PSUM bank: only
                # the first m-tile touching each bank may set it.
                m_per_bank = 512 // (C * BL)
                for m in range(MT):
                    nc.tensor.matmul(
                        xp[:, m],
                        wx_sb[:, 128 * m : 128 * (m + 1)],
                        ft,
                        start=(m % m_per_bank == 0),
                        stop=False,
                        skip_group_check=True,
                    )
                return xp

            for rep in range(repeat):
                nc.sync.dma_start(out=c_t, in_=c0[:, :, :])
                xp_cur = prep_chunk(0)
                for c in range(NCH):
                    ring = rings[c % 2]
                    xp = xp_cur
                    for tl in range(C):
                        t = c * C + tl
                        if t == 0:
                            h_prev = h0_sb[:, :, :]
                        elif tl == 0:
                            h_prev = rings[(c - 1) % 2][:, C - 1]
                        else:
                            h_prev = ring[:, tl - 1]

                        # recurrent matmuls accumulate onto xp in PSUM
                        for m in range(MT):
                            for k in range(KT):
                                nc.tensor.matmul(
                                    xp[:, m, tl],
                                    wh_sb[:, k, 128 * m : 128 * (m + 1)],
                                    h_prev[:, k, :],
                                    start=False,
                                    stop=(k == KT - 1),
                                    skip_group_check=True,
                                )

                        # all-sigmoid nonlinearities (tanh x = 2*sig(2x)-1;
                        # the 2x on g is folded into the weights)
                        sg = sv.tile([128, 4, 2, BL], f32, tag="sg")
                        nc.scalar.activation(sg, xp[:, :, tl], sig_f)
                        tg = sv.tile([128, 2, BL], f32, tag="tg")
                        nc.vector.tensor_scalar(
                            out=tg, in0=sg[:, 3], scalar1=2.0, scalar2=1.0,
                            op0=mult, op1=sub,
                        )
                        fc = sv.tile([128, 2, BL], f32, tag="fc")
                        nc.vector.tensor_mul(fc, sg[:, 1], c_t)
                        ig = sv.tile([128, 2, BL], f32, tag="ig")
                        nc.vector.tensor_mul(ig, sg[:, 0], tg)
                        nc.vector.tensor_add(c_t, fc, ig)
                        sc = sv.tile([128, 2, BL], f32, tag="sc")
                        nc.scalar.activation(sc, c_t, sig_f, scale=2.0)
                        tc_ = sv.tile([128, 2, BL], f32, tag="tc_")
                        nc.vector.tensor_scalar(
                            out=tc_, in0=sc, scalar1=2.0, scalar2=1.0,
                            op0=mult, op1=sub,
                        )
                        nc.vector.tensor_mul(ring[:, tl], sg[:, 2], tc_)

                    if c + 1 < NCH:
                        xp_cur = prep_chunk(c + 1)
                    if rep == repeat - 1:
                        nc.sync.dma_start(out=feat[c], in_=ring)

            nc.sync.dma_start(out=fhh[:, :, :], in_=rings[(NCH - 1) % 2][:, C - 1])
            nc.sync.dma_start(out=fhc[:, :, :], in_=c_t)

    nc.compile()
    return nc


_NC_CACHE = {}


def _get_nc(T, C=8, debug=False, repeat=1):
    key = (T, C, debug, repeat)
    if key not in _NC_CACHE:
        _NC_CACHE[key] = _build(T, C, debug=debug, repeat=repeat)
    return _NC_CACHE[key]


def _pack_inputs(x, hidden, prev_action, prev_reward, prev_done, W_ih, W_hh,
                 b_ih, b_hh, T, C):
    """Host-side packing into the kernel's layouts. Returns per-core in_maps."""
    x = np.asarray(x, dtype=np.float32)
    a = np.asarray(prev_action, dtype=np.float32)
    r = np.asarray(prev_reward, dtype=np.float32)
    d = np.asarray(prev_done, dtype=np.float32)
    hidden = np.asarray(hidden, dtype=np.float32)
    W_ih = np.asarray(W_ih, dtype=np.float32)
    W_hh = np.asarray(W_hh, dtype=np.float32)
    bias = np.asarray(b_ih, dtype=np.float32) + np.asarray(b_hh, dtype=np.float32)

    NCH = T // C
    # m-tile order (i,f,o,g); scale g rows by 2 (tanh x = 2 sig(2x) - 1)
    wh_l = W_hh[_PERM, :].astype(np.float32)
    wh_l[768:] *= 2.0
    wh_l = np.ascontiguousarray(wh_l.T).astype(BF16)  # [256, 1024]
    wx_l = np.zeros((G4, KF), dtype=np.float32)
    wx_l[:, :IN_DIM] = W_ih[_PERM, :]
    wx_l[:, IN_DIM] = bias[_PERM]
    wx_l[768:] *= 2.0
    wx_l = np.ascontiguousarray(wx_l.T).astype(BF16)  # [83, 1024]

    em = np.zeros((5, 80), dtype=np.float32)
    em[0, :NUM_STATES] = 1.0
    em[1, NUM_STATES:NUM_STATES + NUM_ACTIONS] = 1.0

    iota = np.zeros((80, 1), dtype=np.float32)
    iota[:NUM_STATES, 0] = np.arange(NUM_STATES)
    iota[NUM_STATES:80, 0] = np.arange(NUM_ACTIONS)

    in_maps = []
    for i in range(NCORES):
        bs = slice(BL * i, BL * (i + 1))
        inp4 = np.empty((NCH, 5, C * BL), dtype=np.float32)
        for row, src in enumerate((x, a, r, d)):
            v = src[bs, :T].T.reshape(NCH, C, BL)
            inp4[:, row, :] = v.reshape(NCH, C * BL)
        inp4[:, 4, :] = 1.0
        inprd = inp4[:, 2:5, :].astype(BF16)

        hc = hidden[bs]  # [BL, 512] -> [128, 4, BL]
        hc_p = np.ascontiguousarray(hc.reshape(BL, 4, 128).transpose(2, 1, 0))
        h0 = hc_p[:, 0:2].astype(BF16)
        c0 = np.ascontiguousarray(hc_p[:, 2:4])

        in_maps.append({
            "inp4": inp4, "inprd": inprd, "h0": h0, "c0": c0,
            "wh": wh_l, "wx": wx_l, "em": em, "iota": iota,
        })
    return in_maps


def _unpack_outputs(results, T, C):
    features = np.empty((B, T, HIDDEN), dtype=np.float32)
    final_hidden = np.empty((B, 2 * HIDDEN), dtype=np.float32)
    for i, res in enumerate(results):
        bs = slice(BL * i, BL * (i + 1))
        f = np.asarray(res["feat"]).astype(np.float32)  # [NCH, 128, C, 2, BL]
        features[bs] = f.transpose(4, 0, 2, 3, 1).reshape(BL, T, HIDDEN)
        fh = np.asarray(res["fhh"]).astype(np.float32)  # [128, 2, BL]
        fc_ = np.asarray(res["fhc"]).astype(np.float32)
        final_hidden[bs, :HIDDEN] = fh.transpose(2, 1, 0).reshape(BL, HIDDEN)
        final_hidden[bs, HIDDEN:] = fc_.transpose(2, 1, 0).reshape(BL, HIDDEN)
    return features, final_hidden


def kernel(x, hidden, prev_action, prev_reward, prev_done, W_ih, W_hh, b_ih,
           b_hh, _T=None, _C=8):
    from concourse.bass_utils import run_bass_kernel_spmd

    T = _T or T_FULL
    nc = _get_nc(T, _C)
    in_maps = _pack_inputs(x, hidden, prev_action, prev_reward, prev_done,
                           W_ih, W_hh, b_ih, b_hh, T, _C)
    out = run_bass_kernel_spmd(nc, in_maps, core_ids=list(range(NCORES)))
    return _unpack_outputs(out.results, T, _C)
